# revision 20
# baseline (speedup 1.0000x reference)
import os
import numpy as np
import ml_dtypes

# nn_MultiHeadAttention: B=4, S=2048, D=1024, HEADS=16, DIM_HEAD=64.
# Sharding: batch (4) x head-group (2) across 8 cores. Each core computes
# attention for one batch and 8 heads, plus its partial of the output
# projection (row-parallel W0); the two head-group partials per batch are
# summed on the host (bf16 partials, fp32 accumulate).
#
# Schedule: 16 "quanta" (pair p x query-block ibl) processed in wavefront
# order so the projections (q/k/v) and the W0 output projection interleave
# with attention throughout the kernel, keeping both TensorE and the
# scalar (exp) engine busy end-to-end. Within a quantum the score matmuls
# run one key-tile-group ahead of the attention@V matmuls so the exp
# pipeline never starves while AV waits on the previous normalize.
B, S, D = 4, 2048, 1024
HEADS, DH = 16, 64
HPC = 8               # heads per core
E = HPC * DH          # 512 local projection channels
SCALE = DH ** -0.5
P = 128
MT = D // P           # 8 contraction tiles
NPAIR = HPC // 2      # 4 head pairs (= e-chunks of 128)
NTB = S // 512        # 4 query blocks of 512
NJT = S // P          # 16 key tiles of 128
VPW = 65 + 128        # vp columns per pair: h0 [V|1], h1 [1|0*63|V]

_CACHE = {}

# wavefront order over (p, ibl): anti-diagonals of the 4x4 grid
QUANTA = [(p, d - p) for d in range(7) for p in range(4) if 0 <= d - p < 4]


def _build():
    if "nc" in _CACHE:
        return _CACHE["nc"]
    import concourse.bacc as bacc
    import concourse.mybir as mybir
    from concourse.tile import TileContext

    f32 = mybir.dt.float32
    bf16 = mybir.dt.bfloat16
    u16 = mybir.dt.uint16
    EXP = mybir.ActivationFunctionType.Exp
    MULT = mybir.AluOpType.mult
    ADD = mybir.AluOpType.add

    # Schraudolph bf16 exp on the vector engine: uint16(round(A*s + B)) are
    # exactly the bf16 bits of ~exp(SCALE*s) (|rel err| <= ~3%). Offloading
    # a few key tiles per quantum from the scalar engine (the exp bottleneck,
    # ~1.13us per [128,1024] tile) keeps both engines below the tensor
    # engine's per-quantum budget. Verified vs reference in sim: 4 tiles of
    # 16 -> rel_err ~0.009 (gate is 2e-2).
    EXPA = 128.0 / 0.6931471805599453 * SCALE
    EXPB = 127.0 * 128.0 - 5.5
    # early/mid j only: the quantum tail (j>=12) feeds the normalize chain,
    # which shares the vector engine's strict FIFO — offloaded tiles there
    # stall the next quantum's scores
    DVE_EXP_JS = (2, 5, 8, 11)

    nc = bacc.Bacc("TRN2", target_bir_lowering=False, debug=False, num_devices=8)
    qT_d = nc.dram_tensor("qT", [D, S], bf16, kind="ExternalInput")
    kvT_d = nc.dram_tensor("kvT", [D, S], bf16, kind="ExternalInput")
    wq_d = nc.dram_tensor("wqT", [D, E], bf16, kind="ExternalInput")
    wk_d = nc.dram_tensor("wkT", [D, E], bf16, kind="ExternalInput")
    wv_d = nc.dram_tensor("wvT", [D, E], bf16, kind="ExternalInput")
    w0_d = nc.dram_tensor("w0a", [E, D], bf16, kind="ExternalInput")
    out_d = nc.dram_tensor("poutT", [D, S], bf16, kind="ExternalOutput")

    with TileContext(nc) as tc:
        with (
            tc.tile_pool(name="pers", bufs=1) as pers,
            tc.tile_pool(name="psS", bufs=1, space="PSUM") as psS,
            tc.tile_pool(name="psO", bufs=1, space="PSUM") as psO,
            tc.tile_pool(name="psE", bufs=1, space="PSUM") as psE,
        ):
            # ---- persistent SBUF tiles ----
            # packed layouts so each input loads with a single wide DMA:
            #   w*_all cols [mt*E + e]    kv/q_all cols [mt*S + s]
            #   w0_all cols [p*D + d]
            w0_all = pers.tile([P, NPAIR * D], bf16, tag="w0", name="w0")
            qpt = [pers.tile([P, S], bf16, tag=f"qp{p}", name=f"qp{p}") for p in range(NPAIR)]
            kpt = [pers.tile([P, S], bf16, tag=f"kp{p}", name=f"kp{p}") for p in range(NPAIR)]
            vp = [pers.tile([P, NPAIR * VPW], bf16, tag=f"vp{t}", name=f"vp{t}") for t in range(NJT)]
            onorm = [pers.tile([P, S], bf16, tag=f"on{p}", name=f"on{p}") for p in range(NPAIR)]
            q_all = pers.tile([P, MT * S], bf16, tag="qA", name="qA")
            kv_all = pers.tile([P, MT * S], bf16, tag="kvA", name="kvA")
            wq_all = pers.tile([P, MT * E], bf16, tag="wqA", name="wqA")
            wk_all = pers.tile([P, MT * E], bf16, tag="wkA", name="wkA")
            wv_all = pers.tile([P, MT * E], bf16, tag="wvA", name="wvA")

            def wslc(wall, mt, p0, np_=1):
                return wall[:, mt * E + p0 * P: mt * E + (p0 + np_) * P]

            def xslc(xall, mt, c0, c1):
                return xall[:, mt * S + c0: mt * S + c1]

            # ---- DMA emission, deadline-ordered, one wide DMA per chunk ----
            def dma_wall(dst, src):
                d3 = dst.rearrange("x (m e) -> x m e", e=E)
                nc.sync.dma_start(out=d3[:, :, :], in_=src.rearrange("(m x) e -> x m e", x=P))

            def dma_xall(dst, src, tb):
                d3 = dst.rearrange("x (m s) -> x m s", s=S)
                nc.sync.dma_start(
                    out=d3[:, :, tb * 512:(tb + 1) * 512],
                    in_=src.rearrange("(m x) s -> x m s", x=P)[:, :, tb * 512:(tb + 1) * 512])

            # fill-critical slices only: pair-0 weight columns (256KB each)
            # and the tb0 input blocks. Everything else is DMA'd after the
            # fill compute is emitted, so the fill can never serialize on it.
            def dma_wcols(dst, src, e0, e1):
                d3 = dst.rearrange("x (m e) -> x m e", e=E)
                s3 = src.rearrange("(m x) e -> x m e", x=P)
                nc.sync.dma_start(out=d3[:, :, e0:e1], in_=s3[:, :, e0:e1])

            def dma_x_half(dst, src, h, tb):
                d3 = dst.rearrange("x (m s) -> x m s", s=S)
                s3 = src.rearrange("(m x) s -> x m s", x=P)
                nc.sync.dma_start(
                    out=d3[:, 4 * h:4 * (h + 1), tb * 512:(tb + 1) * 512],
                    in_=s3[:, 4 * h:4 * (h + 1), tb * 512:(tb + 1) * 512])

            def dma_wcols_mt(dst, src, e0, e1, m0, m1):
                d3 = dst.rearrange("x (m e) -> x m e", e=E)
                s3 = src.rearrange("(m x) e -> x m e", x=P)
                nc.sync.dma_start(out=d3[:, m0:m1, e0:e1], in_=s3[:, m0:m1, e0:e1])

            def dma_x_mts(dst, src, tb, m0, m1):
                d3 = dst.rearrange("x (m s) -> x m s", s=S)
                s3 = src.rearrange("(m x) s -> x m s", x=P)
                nc.sync.dma_start(
                    out=d3[:, m0:m1, tb * 512:(tb + 1) * 512],
                    in_=s3[:, m0:m1, tb * 512:(tb + 1) * 512])

            # tiny first chunks so kproj's mt0/mt1 matmuls start ASAP
            dma_wcols_mt(wk_all, wk_d, 0, P, 0, 2)
            dma_x_mts(kv_all, kvT_d, 0, 0, 2)
            dma_wcols_mt(wk_all, wk_d, 0, P, 2, MT)
            dma_x_mts(kv_all, kvT_d, 0, 2, MT)
            dma_wcols(wq_all, wq_d, 0, P)
            dma_x_half(q_all, qT_d, 0, 0)
            dma_x_half(q_all, qT_d, 1, 0)

            def dma_rest():
                # wv first: the vp extras woven into quantum 0's early slots
                # head-of-line block the tensor queue until it lands
                dma_wall(wv_all, wv_d)
                dma_xall(kv_all, kvT_d, 1)
                dma_xall(kv_all, kvT_d, 2)
                dma_xall(kv_all, kvT_d, 3)
                dma_wcols(wk_all, wk_d, P, E)
                dma_wcols(wq_all, wq_d, P, E)
                dma_xall(q_all, qT_d, 1)
                dma_xall(q_all, qT_d, 2)
                dma_xall(q_all, qT_d, 3)
                w03 = w0_all.rearrange("x (g c) -> x g c", c=D)
                nc.sync.dma_start(out=w03[:, :, :],
                                  in_=w0_d.rearrange("(g x) c -> x g c", x=P))

            with (
                tc.tile_pool(name="at", bufs=8) as atp,
                tc.tile_pool(name="small", bufs=1) as small,
                tc.tile_pool(name="ob", bufs=3) as obp,
            ):
                # PSUM bank plan (8 banks): sAB 2x[128,1024] (4), plus two
                # pairs of accumulator banks that alternate roles by quantum
                # parity: the active pair holds po0/po1, the other pair is
                # borrowed by projection/W0 "extras" emitted that quantum.
                PAIRS = (("o0", "o1"), ("e0", "e1"))
                cur_parity = [0]     # parity of the quantum being emitted
                ext_tog = [0]
                in_tail = [False]    # after the last quantum all 4 banks free

                def ext_psum(cols=512):
                    if in_tail[0]:
                        tag = ("o0", "o1", "e0", "e1")[ext_tog[0] % 4]
                        ext_tog[0] = (ext_tog[0] + 1) % 4
                    else:
                        pair = PAIRS[1 - cur_parity[0]]
                        tag = pair[ext_tog[0] % 2]
                        ext_tog[0] = (ext_tog[0] + 1) % 2
                    pool = psO if tag[0] == "o" else psE
                    return pool.tile([P, cols], f32, tag=tag, name=tag, bufs=1)

                # ---- work units (each: psum borrow + MMs + copy out) ----
                def warm_mm():
                    # junk matmul on not-yet-written SBUF: keeps the HAM
                    # activity window busy so the PE stays at 2.4GHz across
                    # DMA-wait gaps (no data deps, runs immediately)
                    wps = psS.tile([P, 1024], f32, tag="sAB", name="warm", bufs=2)
                    nc.tensor.matmul(
                        wps[:, 0:512], lhsT=onorm[0][:, 0:128],
                        rhs=onorm[1][:, 0:512], start=True, stop=True,
                        skip_group_check=True)

                def kproj(p, tb, pad=False):
                    ps = ext_psum()
                    for mt in range(MT):
                        nc.tensor.matmul(
                            ps[:, :], lhsT=wslc(wk_all, mt, p),
                            rhs=xslc(kv_all, mt, tb * 512, (tb + 1) * 512),
                            start=(mt == 0), stop=(mt == MT - 1))
                        if pad and mt < MT - 1:
                            warm_mm()
                    nc.vector.tensor_copy(out=kpt[p][:, tb * 512:(tb + 1) * 512], in_=ps[:, :])

                def qproj(p, tb, pad=False):
                    ps = ext_psum()
                    for mt in range(MT):
                        nc.tensor.matmul(
                            ps[:, :], lhsT=wslc(wq_all, mt, p),
                            rhs=xslc(q_all, mt, tb * 512, (tb + 1) * 512),
                            start=(mt == 0), stop=(mt == MT - 1))
                        if pad and mt < MT - 1:
                            warm_mm()
                    nc.vector.tensor_copy(out=qpt[p][:, tb * 512:(tb + 1) * 512], in_=ps[:, :])

                def vp_init(t):
                    nc.gpsimd.memset(vp[t][:, :], 0.0)
                    v3 = vp[t].rearrange("x (g c) -> x g c", c=VPW)
                    nc.gpsimd.memset(v3[:, :, 64:66], 1.0)

                def vp_unit(t, p0, np_):
                    # V projection for pairs [p0, p0+np_) of key tile t
                    ps = ext_psum(np_ * P)
                    for mt in range(MT):
                        nc.tensor.matmul(
                            ps[:, :], lhsT=xslc(kv_all, mt, t * P, (t + 1) * P),
                            rhs=wslc(wv_all, mt, p0, np_),
                            start=(mt == 0), stop=(mt == MT - 1))
                    v3 = vp[t].rearrange("x (g c) -> x g c", c=VPW)
                    p3 = ps.rearrange("x (g c) -> x g c", c=P)
                    nc.vector.tensor_copy(out=v3[:, p0:p0 + np_, 0:64], in_=p3[:, :, 0:64])
                    nc.vector.tensor_copy(out=v3[:, p0:p0 + np_, 129:193], in_=p3[:, :, 64:128])

                def w0_unit(dc, tb):
                    ps = ext_psum()
                    for p in range(NPAIR):
                        nc.tensor.matmul(
                            ps[:, :], lhsT=w0_all[:, p * D + dc * P: p * D + (dc + 1) * P],
                            rhs=onorm[p][:, tb * 512:(tb + 1) * 512],
                            start=(p == 0), stop=(p == NPAIR - 1))
                    ob = obp.tile([P, 512], bf16, tag="ob", name="ob")
                    nc.vector.tensor_copy(out=ob[:, :], in_=ps[:, :])
                    nc.sync.dma_start(
                        out=out_d[dc * P:(dc + 1) * P, tb * 512:(tb + 1) * 512],
                        in_=ob[:, :])

                # ---- extras queue: deadline-ordered list of closures ----
                # Each entry: (deadline_quantum, jg_slot_hint, early_ok, fn).
                # Deadline (dq, djg) means: must be emitted by the drain call
                # at quantum dq, jg slot djg+2. early_ok units may be pulled
                # one quantum ahead to fill slack; W0 units must not be (they
                # would head-of-line block the tensor queue waiting on onorm).
                extras = []

                def build_extras():
                    first_q = {p: min(i for i, (pp, _) in enumerate(QUANTA) if pp == p)
                               for p in range(NPAIR)}
                    # kproj(p, tb): used from jg=2*tb of quantum first_q[p];
                    # tb=0 units land late in the preceding quantum
                    for p in range(NPAIR):
                        for tb in range(NTB):
                            if p == 0 and tb == 0:
                                continue  # in fill
                            dq, djg = (first_q[p], 2 * tb - 2)
                            if tb == 0:
                                dq, djg = first_q[p] - 1, 2
                            extras.append((dq, djg, True,
                                           lambda p=p, tb=tb: kproj(p, tb)))
                    # qproj(p, ibl): used from jg=0 of quantum (p, ibl);
                    # emit late in the preceding quantum
                    qidx = {q: i for i, q in enumerate(QUANTA)}
                    for p in range(NPAIR):
                        for ibl in range(NTB):
                            if p == 0 and ibl == 0:
                                continue
                            extras.append((qidx[(p, ibl)] - 1, 3, True,
                                           lambda p=p, ibl=ibl: qproj(p, ibl)))
                    # vp units: pair 0 just-in-time in quantum first_q[0],
                    # pair 1 by first_q[1], pairs 2-3 by first_q[2] — spread
                    # so no single quantum's tensor queue balloons (the
                    # in-order queue would delay the next quantum's scores).
                    # Pair-0 t=0,1 go first at (0,-1) — after scores j0/j1,
                    # so the first exps are not queued behind them.
                    for t in range(2):
                        extras.append((first_q[0], -1, True,
                                       lambda t=t: vp_unit(t, 0, 1)))
                    for t in range(NJT):
                        if t >= 2:
                            extras.append((first_q[0], t // 2 - 2, True,
                                           lambda t=t: vp_unit(t, 0, 1)))
                        extras.append((first_q[1], t // 2 - 2, True,
                                       lambda t=t: vp_unit(t, 1, 1)))
                        extras.append((first_q[2], t // 2 - 2, True,
                                       lambda t=t: vp_unit(t, 2, 2)))
                    # W0(dc, tb): ready after the last quantum of ibl=tb.
                    # The tb whose last quantum is the final one is handled
                    # by the pipelined tail instead.
                    ready = {ibl: max(i for i, (_, bb) in enumerate(QUANTA) if bb == ibl)
                             for ibl in range(NTB)}
                    for tb in range(NTB):
                        if ready[tb] >= len(QUANTA) - 1:
                            continue
                        for dc in range(D // P):
                            extras.append((ready[tb] + 1 + dc // 4, (dc % 4) * 2, False,
                                           lambda dc=dc, tb=tb: w0_unit(dc, tb)))
                    extras.sort(key=lambda e: (e[0], e[1]))

                build_extras()
                ei = [0]

                def drain_extras(qi, jg, budget=2):
                    # emit every extra whose deadline is before/at this slot,
                    # plus up to `budget` early units within 1 quantum of due
                    emitted_early = 0
                    while ei[0] < len(extras):
                        dq, djg, early_ok, fn = extras[ei[0]]
                        due = (dq, djg) <= (qi, jg)
                        if due:
                            fn()
                            ei[0] += 1
                        elif (early_ok and emitted_early < budget
                              and dq - qi <= 1):
                            fn()
                            ei[0] += 1
                            emitted_early += 1
                        else:
                            break

                # ---- fill ----
                # warmup: the fill is DMA-bound (~2.5MB must land before the
                # first scores), so the PE idles in bursts and HAM holds the
                # array at 1.2GHz. Junk matmuls on not-yet-written SBUF keep
                # the activity window busy so the fill+first-quantum matmuls
                # run at 2.4GHz. onorm is not DMA'd, so no WAR stall.
                for w in range(24):
                    warm_mm()
                for t in range(NJT):
                    vp_init(t)
                kproj(0, 0)
                qproj(0, 0)
                dma_rest()

                # ---- quanta (conveyor: AV runs one key tile behind exp and
                # flows across quantum boundaries; po banks alternate parity)
                pend = []            # deferred av closures (FIFO)

                def flush_pend(keep=0):
                    while len(pend) > keep:
                        pend.pop(0)()

                def normalize(p, ibl, po0, po1):
                    # onorm[e, i] = po[e, i] / sums[i]
                    srow0 = small.tile([1, 512], f32, tag="srow0", name="srow0")
                    srow1 = small.tile([1, 512], f32, tag="srow1", name="srow1")
                    # scalar engine (idle at quantum end) so the DVE FIFO
                    # isn't in the normalize critical path twice
                    nc.scalar.copy(out=srow0[:, :], in_=po0[64:65, :])
                    nc.scalar.copy(out=srow1[:, :], in_=po1[0:1, :])
                    rrow0 = small.tile([1, 512], f32, tag="rrow0", name="rrow0")
                    rrow1 = small.tile([1, 512], f32, tag="rrow1", name="rrow1")
                    nc.vector.reciprocal_approx_fast(out=rrow0[:, :], in_=srow0[:, :])
                    nc.vector.reciprocal_approx_fast(out=rrow1[:, :], in_=srow1[:, :])
                    rbs = small.tile([P, 512], f32, tag="rbs", name="rbs")
                    rbt = small.tile([64, 512], f32, tag="rbt", name="rbt")
                    nc.gpsimd.partition_broadcast(rbs[0:64, :], rrow0[0:1, :], channels=64)
                    nc.gpsimd.partition_broadcast(rbt[0:64, :], rrow1[0:1, :], channels=64)
                    nc.sync.dma_start(out=rbs[64:128, :], in_=rbt[0:64, :])
                    nc.vector.tensor_tensor(
                        out=onorm[p][0:64, ibl * 512:(ibl + 1) * 512],
                        in0=po0[0:64, :], in1=rbs[0:64, :], op=MULT)
                    nc.vector.tensor_tensor(
                        out=onorm[p][64:128, ibl * 512:(ibl + 1) * 512],
                        in0=po1[64:128, :], in1=rbs[64:128, :], op=MULT)

                for qi, (p, ibl) in enumerate(QUANTA):
                    cur_parity[0] = qi % 2
                    tags = PAIRS[cur_parity[0]]
                    pool0 = psO if tags[0][0] == "o" else psE
                    po0 = pool0.tile([65, 512], f32, tag=tags[0], name="po0", bufs=1)
                    po1 = pool0.tile([P, 512], f32, tag=tags[1], name="po1", bufs=1)
                    q0 = qpt[p]
                    vslc0 = (p * VPW, p * VPW + 65)
                    vslc1 = (p * VPW + 65, (p + 1) * VPW)

                    def av(j, at, half, po0=po0, po1=po1, v0=vslc0, v1=vslc1):
                        if half == 0:
                            nc.tensor.matmul(
                                po0[:, :], lhsT=vp[j][:, v0[0]:v0[1]],
                                rhs=at[:, 0:512],
                                start=(j == 0), stop=(j == NJT - 1))
                        else:
                            nc.tensor.matmul(
                                po1[:, :], lhsT=vp[j][:, v1[0]:v1[1]],
                                rhs=at[:, 512:1024],
                                start=(j == 0), stop=(j == NJT - 1))

                    for jg in range(NJT // 2):
                        js = (2 * jg, 2 * jg + 1)
                        ats = []
                        for j in js:
                            sAB = psS.tile([P, 1024], f32, tag="sAB", name="sAB", bufs=2)
                            nc.tensor.matmul(
                                sAB[:, 0:512],
                                lhsT=kpt[p][0:64, j * P:(j + 1) * P],
                                rhs=q0[0:64, ibl * 512:(ibl + 1) * 512],
                                start=True, stop=True,
                                tile_position=(0, 0))
                            nc.tensor.matmul(
                                sAB[:, 512:1024],
                                lhsT=kpt[p][64:128, j * P:(j + 1) * P],
                                rhs=q0[64:128, ibl * 512:(ibl + 1) * 512],
                                start=True, stop=True,
                                tile_position=(64, 0))
                            at = atp.tile([P, 1024], bf16, tag="at", name="at")
                            if j in DVE_EXP_JS:
                                nc.vector.tensor_scalar(
                                    out=at[:, :].bitcast(u16), in0=sAB[:, :],
                                    scalar1=EXPA, scalar2=EXPB,
                                    op0=MULT, op1=ADD)
                            else:
                                nc.scalar.activation(at[:, :], sAB[:, :], EXP, scale=SCALE)
                            ats.append(at)
                        # conveyor: flush deferred AVs (may carry the previous
                        # quantum's normalize), then extras, then defer this
                        # slot's AVs. Quantum 0 runs the AVs three slots
                        # behind so the scores/exp pipeline is not blocked
                        # behind the V-projection weight DMA.
                        # drain the conveyor fully at each quantum boundary so
                        # the previous normalize is emitted before any extras
                        # that wait on the banks it releases
                        keep = 4 if qi == 0 else (0 if jg == 0 else 2)
                        flush_pend(keep=keep)
                        drain_extras(qi, jg - 1)
                        pend.append(lambda j=js[0], at=ats[0], f=av:
                                    (f(j, at, 0), f(j, at, 1)))
                        if js[1] == NJT - 1:
                            pend.append(lambda j=js[1], at=ats[1], p=p, ibl=ibl,
                                        po0=po0, po1=po1, f=av:
                                        (f(j, at, 0), f(j, at, 1),
                                         normalize(p, ibl, po0, po1)))
                        else:
                            pend.append(lambda j=js[1], at=ats[1], f=av:
                                        (f(j, at, 0), f(j, at, 1)))

                def w0_tail(tb):
                    # software-pipelined final W0 block: pairs 0-2 of each
                    # dc accumulate before the last normalize lands; only
                    # the pair-3 matmul waits on it. Two waves of 4 banks.
                    for wave in range(2):
                        states = []
                        for dc in range(4 * wave, 4 * wave + 4):
                            ps = ext_psum()
                            for pi in range(NPAIR - 1):
                                nc.tensor.matmul(
                                    ps[:, :],
                                    lhsT=w0_all[:, pi * D + dc * P: pi * D + (dc + 1) * P],
                                    rhs=onorm[pi][:, tb * 512:(tb + 1) * 512],
                                    start=(pi == 0), stop=False)
                            states.append((dc, ps))
                        for i, (dc, ps) in enumerate(states):
                            nc.tensor.matmul(
                                ps[:, :],
                                lhsT=w0_all[:, 3 * D + dc * P: 3 * D + (dc + 1) * P],
                                rhs=onorm[3][:, tb * 512:(tb + 1) * 512],
                                start=False, stop=True)
                            ob = obp.tile([P, 512], bf16, tag="ob", name="ob")
                            # drain-critical: alternate evacuation between the
                            # (idle) scalar engine and vector, and issue each
                            # DMA from that engine's own queue so the Sync
                            # queue's ~0.9us per-DMA issue cost stops
                            # serializing the tail.
                            if i % 2 == 0:
                                nc.scalar.copy(out=ob[:, :], in_=ps[:, :])
                                nc.scalar.dma_start(
                                    out=out_d[dc * P:(dc + 1) * P, tb * 512:(tb + 1) * 512],
                                    in_=ob[:, :])
                            else:
                                nc.vector.tensor_copy(out=ob[:, :], in_=ps[:, :])
                                nc.sync.dma_start(
                                    out=out_d[dc * P:(dc + 1) * P, tb * 512:(tb + 1) * 512],
                                    in_=ob[:, :])

                # last quantum's final AV + normalize, then remaining extras
                flush_pend()
                in_tail[0] = True
                drain_extras(10 ** 9, 10 ** 9)
                tail_tb = max(range(NTB), key=lambda ibl: max(
                    i for i, (_, bb) in enumerate(QUANTA) if bb == ibl))
                w0_tail(tail_tb)

    nc.compile()
    _CACHE["nc"] = nc
    return nc


def _prep_weights(Wq, Wkv, W0):
    bf = ml_dtypes.bfloat16
    per_group = {}
    for g in range(2):
        hg = np.arange(HPC) + g * HPC            # global head ids
        d = np.arange(DH)
        # e_local = h_l*64 + d ; reference maps: e_q = d*16+h, e_k = d*32+h,
        # e_v = d*32+16+h, out channel = h*64+d
        idx_q = (d[None, :] * HEADS + hg[:, None]).reshape(-1)
        idx_k = (d[None, :] * 2 * HEADS + hg[:, None]).reshape(-1)
        idx_v = (d[None, :] * 2 * HEADS + HEADS + hg[:, None]).reshape(-1)
        idx_o = (hg[:, None] * DH + d[None, :]).reshape(-1)
        per_group[g] = {
            "wqT": np.ascontiguousarray(Wq[idx_q, :].T).astype(bf),
            "wkT": np.ascontiguousarray(Wkv[idx_k, :].T).astype(bf),
            "wvT": np.ascontiguousarray(Wkv[idx_v, :].T).astype(bf),
            "w0a": np.ascontiguousarray(W0[:, idx_o].T).astype(bf),
        }
    return per_group


def kernel(q, kv, Wq, Wkv, W0):
    from concourse.bass_utils import run_bass_kernel_spmd

    q = np.asarray(q, dtype=np.float32)
    kv = np.asarray(kv, dtype=np.float32)
    Wq = np.asarray(Wq, dtype=np.float32)
    Wkv = np.asarray(Wkv, dtype=np.float32)
    W0 = np.asarray(W0, dtype=np.float32)

    nc = _build()
    bf = ml_dtypes.bfloat16
    wg = _prep_weights(Wq, Wkv, W0)
    in_maps = []
    for c in range(8):
        b, g = divmod(c, 2)
        in_maps.append({
            "qT": np.ascontiguousarray(q[b].T).astype(bf),
            "kvT": np.ascontiguousarray(kv[b].T).astype(bf),
            "wqT": wg[g]["wqT"],
            "wkT": wg[g]["wkT"],
            "wvT": wg[g]["wvT"],
            "w0a": wg[g]["w0a"],
        })
    trace = bool(int(os.environ.get("KERNEL_TRACE", "0")))
    res = run_bass_kernel_spmd(nc, in_maps, list(range(8)), trace=trace)
    _CACHE["last_result"] = res
    out = np.empty((B, S, D), dtype=np.float32)
    for b in range(B):
        acc = (res.results[2 * b]["poutT"].astype(np.float32)
               + res.results[2 * b + 1]["poutT"].astype(np.float32))
        out[b] = acc.T
    return out



# revision 21
# speedup vs baseline: 1.0007x; 1.0007x over previous
import os
import numpy as np
import ml_dtypes

# nn_MultiHeadAttention: B=4, S=2048, D=1024, HEADS=16, DIM_HEAD=64.
# Sharding: batch (4) x head-group (2) across 8 cores. Each core computes
# attention for one batch and 8 heads, plus its partial of the output
# projection (row-parallel W0); the two head-group partials per batch are
# summed on the host (bf16 partials, fp32 accumulate).
#
# Schedule: 16 "quanta" (pair p x query-block ibl) processed in wavefront
# order so the projections (q/k/v) and the W0 output projection interleave
# with attention throughout the kernel, keeping both TensorE and the
# scalar (exp) engine busy end-to-end. Within a quantum the score matmuls
# run one key-tile-group ahead of the attention@V matmuls so the exp
# pipeline never starves while AV waits on the previous normalize.
B, S, D = 4, 2048, 1024
HEADS, DH = 16, 64
HPC = 8               # heads per core
E = HPC * DH          # 512 local projection channels
SCALE = DH ** -0.5
P = 128
MT = D // P           # 8 contraction tiles
NPAIR = HPC // 2      # 4 head pairs (= e-chunks of 128)
NTB = S // 512        # 4 query blocks of 512
NJT = S // P          # 16 key tiles of 128
VPW = 65 + 128        # vp columns per pair: h0 [V|1], h1 [1|0*63|V]

_CACHE = {}

# wavefront order over (p, ibl): anti-diagonals of the 4x4 grid
QUANTA = [(p, d - p) for d in range(7) for p in range(4) if 0 <= d - p < 4]


def _build():
    if "nc" in _CACHE:
        return _CACHE["nc"]
    import concourse.bacc as bacc
    import concourse.mybir as mybir
    from concourse.tile import TileContext

    f32 = mybir.dt.float32
    bf16 = mybir.dt.bfloat16
    u16 = mybir.dt.uint16
    EXP = mybir.ActivationFunctionType.Exp
    MULT = mybir.AluOpType.mult
    ADD = mybir.AluOpType.add

    # Schraudolph bf16 exp on the vector engine: uint16(round(A*s + B)) are
    # exactly the bf16 bits of ~exp(SCALE*s) (|rel err| <= ~3%). Offloading
    # a few key tiles per quantum from the scalar engine (the exp bottleneck,
    # ~1.13us per [128,1024] tile) keeps both engines below the tensor
    # engine's per-quantum budget. Verified vs reference in sim: 4 tiles of
    # 16 -> rel_err ~0.009 (gate is 2e-2).
    EXPA = 128.0 / 0.6931471805599453 * SCALE
    EXPB = 127.0 * 128.0 - 5.5
    # early/mid j only: the quantum tail (j>=12) feeds the normalize chain,
    # which shares the vector engine's strict FIFO — offloaded tiles there
    # stall the next quantum's scores
    DVE_EXP_JS = (2, 5, 8, 11)

    nc = bacc.Bacc("TRN2", target_bir_lowering=False, debug=False, num_devices=8)
    qT_d = nc.dram_tensor("qT", [D, S], bf16, kind="ExternalInput")
    kvT_d = nc.dram_tensor("kvT", [D, S], bf16, kind="ExternalInput")
    wq_d = nc.dram_tensor("wqT", [D, E], bf16, kind="ExternalInput")
    wk_d = nc.dram_tensor("wkT", [D, E], bf16, kind="ExternalInput")
    wv_d = nc.dram_tensor("wvT", [D, E], bf16, kind="ExternalInput")
    w0_d = nc.dram_tensor("w0a", [E, D], bf16, kind="ExternalInput")
    out_d = nc.dram_tensor("poutT", [D, S], bf16, kind="ExternalOutput")

    with TileContext(nc) as tc:
        with (
            tc.tile_pool(name="pers", bufs=1) as pers,
            tc.tile_pool(name="psS", bufs=1, space="PSUM") as psS,
            tc.tile_pool(name="psO", bufs=1, space="PSUM") as psO,
            tc.tile_pool(name="psE", bufs=1, space="PSUM") as psE,
        ):
            # ---- persistent SBUF tiles ----
            # packed layouts so each input loads with a single wide DMA:
            #   w*_all cols [mt*E + e]    kv/q_all cols [mt*S + s]
            #   w0_all cols [p*D + d]
            w0_all = pers.tile([P, NPAIR * D], bf16, tag="w0", name="w0")
            qpt = [pers.tile([P, S], bf16, tag=f"qp{p}", name=f"qp{p}") for p in range(NPAIR)]
            kpt = [pers.tile([P, S], bf16, tag=f"kp{p}", name=f"kp{p}") for p in range(NPAIR)]
            vp = [pers.tile([P, NPAIR * VPW], bf16, tag=f"vp{t}", name=f"vp{t}") for t in range(NJT)]
            onorm = [pers.tile([P, S], bf16, tag=f"on{p}", name=f"on{p}") for p in range(NPAIR)]
            q_all = pers.tile([P, MT * S], bf16, tag="qA", name="qA")
            kv_all = pers.tile([P, MT * S], bf16, tag="kvA", name="kvA")
            wq_all = pers.tile([P, MT * E], bf16, tag="wqA", name="wqA")
            wk_all = pers.tile([P, MT * E], bf16, tag="wkA", name="wkA")
            wv_all = pers.tile([P, MT * E], bf16, tag="wvA", name="wvA")

            def wslc(wall, mt, p0, np_=1):
                return wall[:, mt * E + p0 * P: mt * E + (p0 + np_) * P]

            def xslc(xall, mt, c0, c1):
                return xall[:, mt * S + c0: mt * S + c1]

            # ---- DMA emission, deadline-ordered, one wide DMA per chunk ----
            def dma_wall(dst, src):
                d3 = dst.rearrange("x (m e) -> x m e", e=E)
                nc.sync.dma_start(out=d3[:, :, :], in_=src.rearrange("(m x) e -> x m e", x=P))

            def dma_xall(dst, src, tb):
                d3 = dst.rearrange("x (m s) -> x m s", s=S)
                nc.sync.dma_start(
                    out=d3[:, :, tb * 512:(tb + 1) * 512],
                    in_=src.rearrange("(m x) s -> x m s", x=P)[:, :, tb * 512:(tb + 1) * 512])

            # fill-critical slices only: pair-0 weight columns (256KB each)
            # and the tb0 input blocks. Everything else is DMA'd after the
            # fill compute is emitted, so the fill can never serialize on it.
            def dma_wcols(dst, src, e0, e1):
                d3 = dst.rearrange("x (m e) -> x m e", e=E)
                s3 = src.rearrange("(m x) e -> x m e", x=P)
                nc.sync.dma_start(out=d3[:, :, e0:e1], in_=s3[:, :, e0:e1])

            def dma_x_half(dst, src, h, tb):
                d3 = dst.rearrange("x (m s) -> x m s", s=S)
                s3 = src.rearrange("(m x) s -> x m s", x=P)
                nc.sync.dma_start(
                    out=d3[:, 4 * h:4 * (h + 1), tb * 512:(tb + 1) * 512],
                    in_=s3[:, 4 * h:4 * (h + 1), tb * 512:(tb + 1) * 512])

            def dma_wcols_mt(dst, src, e0, e1, m0, m1):
                d3 = dst.rearrange("x (m e) -> x m e", e=E)
                s3 = src.rearrange("(m x) e -> x m e", x=P)
                nc.sync.dma_start(out=d3[:, m0:m1, e0:e1], in_=s3[:, m0:m1, e0:e1])

            def dma_x_mts(dst, src, tb, m0, m1):
                d3 = dst.rearrange("x (m s) -> x m s", s=S)
                s3 = src.rearrange("(m x) s -> x m s", x=P)
                nc.sync.dma_start(
                    out=d3[:, m0:m1, tb * 512:(tb + 1) * 512],
                    in_=s3[:, m0:m1, tb * 512:(tb + 1) * 512])

            # tiny first chunks so kproj's mt0/mt1 matmuls start ASAP
            dma_wcols_mt(wk_all, wk_d, 0, P, 0, 2)
            dma_x_mts(kv_all, kvT_d, 0, 0, 2)
            dma_wcols_mt(wk_all, wk_d, 0, P, 2, MT)
            dma_x_mts(kv_all, kvT_d, 0, 2, MT)
            dma_wcols(wq_all, wq_d, 0, P)
            dma_x_half(q_all, qT_d, 0, 0)
            dma_x_half(q_all, qT_d, 1, 0)

            def dma_rest():
                # wv first: the vp extras woven into quantum 0's early slots
                # head-of-line block the tensor queue until it lands
                dma_wall(wv_all, wv_d)
                dma_xall(kv_all, kvT_d, 1)
                dma_xall(kv_all, kvT_d, 2)
                dma_xall(kv_all, kvT_d, 3)
                dma_wcols(wk_all, wk_d, P, E)
                dma_wcols(wq_all, wq_d, P, E)
                dma_xall(q_all, qT_d, 1)
                dma_xall(q_all, qT_d, 2)
                dma_xall(q_all, qT_d, 3)
                w03 = w0_all.rearrange("x (g c) -> x g c", c=D)
                nc.sync.dma_start(out=w03[:, :, :],
                                  in_=w0_d.rearrange("(g x) c -> x g c", x=P))

            with (
                tc.tile_pool(name="at", bufs=8) as atp,
                tc.tile_pool(name="small", bufs=1) as small,
                tc.tile_pool(name="ob", bufs=3) as obp,
            ):
                # PSUM bank plan (8 banks): sAB 2x[128,1024] (4), plus two
                # pairs of accumulator banks that alternate roles by quantum
                # parity: the active pair holds po0/po1, the other pair is
                # borrowed by projection/W0 "extras" emitted that quantum.
                PAIRS = (("o0", "o1"), ("e0", "e1"))
                cur_parity = [0]     # parity of the quantum being emitted
                ext_tog = [0]
                in_tail = [False]    # after the last quantum all 4 banks free

                def ext_psum(cols=512):
                    if in_tail[0]:
                        tag = ("o0", "o1", "e0", "e1")[ext_tog[0] % 4]
                        ext_tog[0] = (ext_tog[0] + 1) % 4
                    else:
                        pair = PAIRS[1 - cur_parity[0]]
                        tag = pair[ext_tog[0] % 2]
                        ext_tog[0] = (ext_tog[0] + 1) % 2
                    pool = psO if tag[0] == "o" else psE
                    return pool.tile([P, cols], f32, tag=tag, name=tag, bufs=1)

                # ---- work units (each: psum borrow + MMs + copy out) ----
                def warm_mm():
                    # junk matmul on not-yet-written SBUF: keeps the HAM
                    # activity window busy so the PE stays at 2.4GHz across
                    # DMA-wait gaps (no data deps, runs immediately)
                    wps = psS.tile([P, 1024], f32, tag="sAB", name="warm", bufs=2)
                    nc.tensor.matmul(
                        wps[:, 0:512], lhsT=onorm[0][:, 0:128],
                        rhs=onorm[1][:, 0:512], start=True, stop=True,
                        skip_group_check=True)

                def kproj(p, tb, pad=False):
                    ps = ext_psum()
                    for mt in range(MT):
                        nc.tensor.matmul(
                            ps[:, :], lhsT=wslc(wk_all, mt, p),
                            rhs=xslc(kv_all, mt, tb * 512, (tb + 1) * 512),
                            start=(mt == 0), stop=(mt == MT - 1))
                        if pad and mt < MT - 1:
                            warm_mm()
                    nc.vector.tensor_copy(out=kpt[p][:, tb * 512:(tb + 1) * 512], in_=ps[:, :])

                def qproj(p, tb, pad=False):
                    ps = ext_psum()
                    for mt in range(MT):
                        nc.tensor.matmul(
                            ps[:, :], lhsT=wslc(wq_all, mt, p),
                            rhs=xslc(q_all, mt, tb * 512, (tb + 1) * 512),
                            start=(mt == 0), stop=(mt == MT - 1))
                        if pad and mt < MT - 1:
                            warm_mm()
                    nc.vector.tensor_copy(out=qpt[p][:, tb * 512:(tb + 1) * 512], in_=ps[:, :])

                def vp_init(t):
                    nc.gpsimd.memset(vp[t][:, :], 0.0)
                    v3 = vp[t].rearrange("x (g c) -> x g c", c=VPW)
                    nc.gpsimd.memset(v3[:, :, 64:66], 1.0)

                def vp_unit(t, p0, np_):
                    # V projection for pairs [p0, p0+np_) of key tile t
                    ps = ext_psum(np_ * P)
                    for mt in range(MT):
                        nc.tensor.matmul(
                            ps[:, :], lhsT=xslc(kv_all, mt, t * P, (t + 1) * P),
                            rhs=wslc(wv_all, mt, p0, np_),
                            start=(mt == 0), stop=(mt == MT - 1))
                    v3 = vp[t].rearrange("x (g c) -> x g c", c=VPW)
                    p3 = ps.rearrange("x (g c) -> x g c", c=P)
                    nc.vector.tensor_copy(out=v3[:, p0:p0 + np_, 0:64], in_=p3[:, :, 0:64])
                    nc.vector.tensor_copy(out=v3[:, p0:p0 + np_, 129:193], in_=p3[:, :, 64:128])

                def w0_unit(dc, tb):
                    ps = ext_psum()
                    for p in range(NPAIR):
                        nc.tensor.matmul(
                            ps[:, :], lhsT=w0_all[:, p * D + dc * P: p * D + (dc + 1) * P],
                            rhs=onorm[p][:, tb * 512:(tb + 1) * 512],
                            start=(p == 0), stop=(p == NPAIR - 1))
                    ob = obp.tile([P, 512], bf16, tag="ob", name="ob")
                    nc.vector.tensor_copy(out=ob[:, :], in_=ps[:, :])
                    nc.sync.dma_start(
                        out=out_d[dc * P:(dc + 1) * P, tb * 512:(tb + 1) * 512],
                        in_=ob[:, :])

                # ---- extras queue: deadline-ordered list of closures ----
                # Each entry: (deadline_quantum, jg_slot_hint, early_ok, fn).
                # Deadline (dq, djg) means: must be emitted by the drain call
                # at quantum dq, jg slot djg+2. early_ok units may be pulled
                # one quantum ahead to fill slack; W0 units must not be (they
                # would head-of-line block the tensor queue waiting on onorm).
                extras = []

                def build_extras():
                    first_q = {p: min(i for i, (pp, _) in enumerate(QUANTA) if pp == p)
                               for p in range(NPAIR)}
                    # kproj(p, tb): used from jg=2*tb of quantum first_q[p];
                    # tb=0 units land late in the preceding quantum
                    for p in range(NPAIR):
                        for tb in range(NTB):
                            if p == 0 and tb == 0:
                                continue  # in fill
                            dq, djg = (first_q[p], 2 * tb - 2)
                            if tb == 0:
                                dq, djg = first_q[p] - 1, 2
                            extras.append((dq, djg, True,
                                           lambda p=p, tb=tb: kproj(p, tb)))
                    # qproj(p, ibl): used from jg=0 of quantum (p, ibl);
                    # emit late in the preceding quantum
                    qidx = {q: i for i, q in enumerate(QUANTA)}
                    for p in range(NPAIR):
                        for ibl in range(NTB):
                            if p == 0 and ibl == 0:
                                continue
                            extras.append((qidx[(p, ibl)] - 1, 3, True,
                                           lambda p=p, ibl=ibl: qproj(p, ibl)))
                    # vp units: pair 0 just-in-time in quantum first_q[0],
                    # pair 1 by first_q[1], pairs 2-3 by first_q[2] — spread
                    # so no single quantum's tensor queue balloons (the
                    # in-order queue would delay the next quantum's scores).
                    # Pair-0 t=0,1 go first at (0,-1) — after scores j0/j1,
                    # so the first exps are not queued behind them.
                    for t in range(2):
                        extras.append((first_q[0], -1, True,
                                       lambda t=t: vp_unit(t, 0, 1)))
                    for t in range(NJT):
                        if t >= 2:
                            extras.append((first_q[0], t // 2 - 2, True,
                                           lambda t=t: vp_unit(t, 0, 1)))
                        extras.append((first_q[1], t // 2 - 2, True,
                                       lambda t=t: vp_unit(t, 1, 1)))
                        extras.append((first_q[2], t // 2 - 2, True,
                                       lambda t=t: vp_unit(t, 2, 2)))
                    # W0(dc, tb): ready after the last quantum of ibl=tb.
                    # The tb whose last quantum is the final one is handled
                    # by the pipelined tail instead.
                    ready = {ibl: max(i for i, (_, bb) in enumerate(QUANTA) if bb == ibl)
                             for ibl in range(NTB)}
                    for tb in range(NTB):
                        if ready[tb] >= len(QUANTA) - 1:
                            continue
                        for dc in range(D // P):
                            extras.append((ready[tb] + 1 + dc // 4, (dc % 4) * 2, False,
                                           lambda dc=dc, tb=tb: w0_unit(dc, tb)))
                    extras.sort(key=lambda e: (e[0], e[1]))

                build_extras()
                ei = [0]

                def drain_extras(qi, jg, budget=1):
                    # emit every extra whose deadline is before/at this slot,
                    # plus up to `budget` early units within 1 quantum of due
                    emitted_early = 0
                    while ei[0] < len(extras):
                        dq, djg, early_ok, fn = extras[ei[0]]
                        due = (dq, djg) <= (qi, jg)
                        if due:
                            fn()
                            ei[0] += 1
                        elif (early_ok and emitted_early < budget
                              and dq - qi <= 1):
                            fn()
                            ei[0] += 1
                            emitted_early += 1
                        else:
                            break

                # ---- fill ----
                # warmup: the fill is DMA-bound (~2.5MB must land before the
                # first scores), so the PE idles in bursts and HAM holds the
                # array at 1.2GHz. Junk matmuls on not-yet-written SBUF keep
                # the activity window busy so the fill+first-quantum matmuls
                # run at 2.4GHz. onorm is not DMA'd, so no WAR stall.
                for w in range(24):
                    warm_mm()
                for t in range(NJT):
                    vp_init(t)
                kproj(0, 0)
                qproj(0, 0)
                dma_rest()

                # ---- quanta (conveyor: AV runs one key tile behind exp and
                # flows across quantum boundaries; po banks alternate parity)
                pend = []            # deferred av closures (FIFO)

                def flush_pend(keep=0):
                    while len(pend) > keep:
                        pend.pop(0)()

                def normalize(p, ibl, po0, po1):
                    # onorm[e, i] = po[e, i] / sums[i]
                    srow0 = small.tile([1, 512], f32, tag="srow0", name="srow0")
                    srow1 = small.tile([1, 512], f32, tag="srow1", name="srow1")
                    # scalar engine (idle at quantum end) so the DVE FIFO
                    # isn't in the normalize critical path twice
                    nc.scalar.copy(out=srow0[:, :], in_=po0[64:65, :])
                    nc.scalar.copy(out=srow1[:, :], in_=po1[0:1, :])
                    rrow0 = small.tile([1, 512], f32, tag="rrow0", name="rrow0")
                    rrow1 = small.tile([1, 512], f32, tag="rrow1", name="rrow1")
                    nc.vector.reciprocal_approx_fast(out=rrow0[:, :], in_=srow0[:, :])
                    nc.vector.reciprocal_approx_fast(out=rrow1[:, :], in_=srow1[:, :])
                    rbs = small.tile([P, 512], f32, tag="rbs", name="rbs")
                    rbt = small.tile([64, 512], f32, tag="rbt", name="rbt")
                    nc.gpsimd.partition_broadcast(rbs[0:64, :], rrow0[0:1, :], channels=64)
                    nc.gpsimd.partition_broadcast(rbt[0:64, :], rrow1[0:1, :], channels=64)
                    nc.sync.dma_start(out=rbs[64:128, :], in_=rbt[0:64, :])
                    nc.vector.tensor_tensor(
                        out=onorm[p][0:64, ibl * 512:(ibl + 1) * 512],
                        in0=po0[0:64, :], in1=rbs[0:64, :], op=MULT)
                    nc.vector.tensor_tensor(
                        out=onorm[p][64:128, ibl * 512:(ibl + 1) * 512],
                        in0=po1[64:128, :], in1=rbs[64:128, :], op=MULT)

                for qi, (p, ibl) in enumerate(QUANTA):
                    cur_parity[0] = qi % 2
                    tags = PAIRS[cur_parity[0]]
                    pool0 = psO if tags[0][0] == "o" else psE
                    po0 = pool0.tile([65, 512], f32, tag=tags[0], name="po0", bufs=1)
                    po1 = pool0.tile([P, 512], f32, tag=tags[1], name="po1", bufs=1)
                    q0 = qpt[p]
                    vslc0 = (p * VPW, p * VPW + 65)
                    vslc1 = (p * VPW + 65, (p + 1) * VPW)

                    def av(j, at, half, po0=po0, po1=po1, v0=vslc0, v1=vslc1):
                        if half == 0:
                            nc.tensor.matmul(
                                po0[:, :], lhsT=vp[j][:, v0[0]:v0[1]],
                                rhs=at[:, 0:512],
                                start=(j == 0), stop=(j == NJT - 1))
                        else:
                            nc.tensor.matmul(
                                po1[:, :], lhsT=vp[j][:, v1[0]:v1[1]],
                                rhs=at[:, 512:1024],
                                start=(j == 0), stop=(j == NJT - 1))

                    for jg in range(NJT // 2):
                        js = (2 * jg, 2 * jg + 1)
                        ats = []
                        for j in js:
                            sAB = psS.tile([P, 1024], f32, tag="sAB", name="sAB", bufs=2)
                            nc.tensor.matmul(
                                sAB[:, 0:512],
                                lhsT=kpt[p][0:64, j * P:(j + 1) * P],
                                rhs=q0[0:64, ibl * 512:(ibl + 1) * 512],
                                start=True, stop=True,
                                tile_position=(0, 0))
                            nc.tensor.matmul(
                                sAB[:, 512:1024],
                                lhsT=kpt[p][64:128, j * P:(j + 1) * P],
                                rhs=q0[64:128, ibl * 512:(ibl + 1) * 512],
                                start=True, stop=True,
                                tile_position=(64, 0))
                            at = atp.tile([P, 1024], bf16, tag="at", name="at")
                            if j in DVE_EXP_JS:
                                nc.vector.tensor_scalar(
                                    out=at[:, :].bitcast(u16), in0=sAB[:, :],
                                    scalar1=EXPA, scalar2=EXPB,
                                    op0=MULT, op1=ADD)
                            else:
                                nc.scalar.activation(at[:, :], sAB[:, :], EXP, scale=SCALE)
                            ats.append(at)
                        # conveyor: flush deferred AVs (may carry the previous
                        # quantum's normalize), then extras, then defer this
                        # slot's AVs. Quantum 0 runs the AVs three slots
                        # behind so the scores/exp pipeline is not blocked
                        # behind the V-projection weight DMA.
                        # drain the conveyor fully at each quantum boundary so
                        # the previous normalize is emitted before any extras
                        # that wait on the banks it releases
                        keep = 4 if qi == 0 else (0 if jg == 0 else 2)
                        flush_pend(keep=keep)
                        drain_extras(qi, jg - 1)
                        pend.append(lambda j=js[0], at=ats[0], f=av:
                                    (f(j, at, 0), f(j, at, 1)))
                        if js[1] == NJT - 1:
                            pend.append(lambda j=js[1], at=ats[1], p=p, ibl=ibl,
                                        po0=po0, po1=po1, f=av:
                                        (f(j, at, 0), f(j, at, 1),
                                         normalize(p, ibl, po0, po1)))
                        else:
                            pend.append(lambda j=js[1], at=ats[1], f=av:
                                        (f(j, at, 0), f(j, at, 1)))

                def w0_tail(tb):
                    # software-pipelined final W0 block: pairs 0-2 of each
                    # dc accumulate before the last normalize lands; only
                    # the pair-3 matmul waits on it. Two waves of 4 banks.
                    for wave in range(2):
                        states = []
                        for dc in range(4 * wave, 4 * wave + 4):
                            ps = ext_psum()
                            for pi in range(NPAIR - 1):
                                nc.tensor.matmul(
                                    ps[:, :],
                                    lhsT=w0_all[:, pi * D + dc * P: pi * D + (dc + 1) * P],
                                    rhs=onorm[pi][:, tb * 512:(tb + 1) * 512],
                                    start=(pi == 0), stop=False)
                            states.append((dc, ps))
                        for i, (dc, ps) in enumerate(states):
                            nc.tensor.matmul(
                                ps[:, :],
                                lhsT=w0_all[:, 3 * D + dc * P: 3 * D + (dc + 1) * P],
                                rhs=onorm[3][:, tb * 512:(tb + 1) * 512],
                                start=False, stop=True)
                            ob = obp.tile([P, 512], bf16, tag="ob", name="ob")
                            # drain-critical: alternate evacuation between the
                            # (idle) scalar engine and vector, and issue each
                            # DMA from that engine's own queue so the Sync
                            # queue's ~0.9us per-DMA issue cost stops
                            # serializing the tail.
                            if i % 2 == 0:
                                nc.scalar.copy(out=ob[:, :], in_=ps[:, :])
                                nc.scalar.dma_start(
                                    out=out_d[dc * P:(dc + 1) * P, tb * 512:(tb + 1) * 512],
                                    in_=ob[:, :])
                            else:
                                nc.vector.tensor_copy(out=ob[:, :], in_=ps[:, :])
                                nc.sync.dma_start(
                                    out=out_d[dc * P:(dc + 1) * P, tb * 512:(tb + 1) * 512],
                                    in_=ob[:, :])

                # last quantum's final AV + normalize, then remaining extras
                flush_pend()
                in_tail[0] = True
                drain_extras(10 ** 9, 10 ** 9)
                tail_tb = max(range(NTB), key=lambda ibl: max(
                    i for i, (_, bb) in enumerate(QUANTA) if bb == ibl))
                w0_tail(tail_tb)

    nc.compile()
    _CACHE["nc"] = nc
    return nc


def _prep_weights(Wq, Wkv, W0):
    bf = ml_dtypes.bfloat16
    per_group = {}
    for g in range(2):
        hg = np.arange(HPC) + g * HPC            # global head ids
        d = np.arange(DH)
        # e_local = h_l*64 + d ; reference maps: e_q = d*16+h, e_k = d*32+h,
        # e_v = d*32+16+h, out channel = h*64+d
        idx_q = (d[None, :] * HEADS + hg[:, None]).reshape(-1)
        idx_k = (d[None, :] * 2 * HEADS + hg[:, None]).reshape(-1)
        idx_v = (d[None, :] * 2 * HEADS + HEADS + hg[:, None]).reshape(-1)
        idx_o = (hg[:, None] * DH + d[None, :]).reshape(-1)
        per_group[g] = {
            "wqT": np.ascontiguousarray(Wq[idx_q, :].T).astype(bf),
            "wkT": np.ascontiguousarray(Wkv[idx_k, :].T).astype(bf),
            "wvT": np.ascontiguousarray(Wkv[idx_v, :].T).astype(bf),
            "w0a": np.ascontiguousarray(W0[:, idx_o].T).astype(bf),
        }
    return per_group


def kernel(q, kv, Wq, Wkv, W0):
    from concourse.bass_utils import run_bass_kernel_spmd

    q = np.asarray(q, dtype=np.float32)
    kv = np.asarray(kv, dtype=np.float32)
    Wq = np.asarray(Wq, dtype=np.float32)
    Wkv = np.asarray(Wkv, dtype=np.float32)
    W0 = np.asarray(W0, dtype=np.float32)

    nc = _build()
    bf = ml_dtypes.bfloat16
    wg = _prep_weights(Wq, Wkv, W0)
    in_maps = []
    for c in range(8):
        b, g = divmod(c, 2)
        in_maps.append({
            "qT": np.ascontiguousarray(q[b].T).astype(bf),
            "kvT": np.ascontiguousarray(kv[b].T).astype(bf),
            "wqT": wg[g]["wqT"],
            "wkT": wg[g]["wkT"],
            "wvT": wg[g]["wvT"],
            "w0a": wg[g]["w0a"],
        })
    trace = bool(int(os.environ.get("KERNEL_TRACE", "0")))
    res = run_bass_kernel_spmd(nc, in_maps, list(range(8)), trace=trace)
    _CACHE["last_result"] = res
    out = np.empty((B, S, D), dtype=np.float32)
    for b in range(B):
        acc = (res.results[2 * b]["poutT"].astype(np.float32)
               + res.results[2 * b + 1]["poutT"].astype(np.float32))
        out[b] = acc.T
    return out

